# revision 27
# baseline (speedup 1.0000x reference)
"""Ball attention (block-local attention, ball size 128) on 8 Trainium2 cores.

Reference computation (per (b,h) head, per ball of 128 consecutive tokens):
    S = Q K^T / sqrt(64);  P = softmax(S, axis=-1);  O = P V

Sharding: the 64 (b,h) heads are split 8-per-core (pure data parallel).

Memory-roofline design (363us baseline -> ~114us measured):
  * All device I/O in fp16 (HBM traffic 33.7 MB/core vs 67 MB in fp32;
    keeps ~7e-4 output error, far under the 2e-2 gate). 33.7 MB at the
    358 GB/s per-core HBM limit is a ~94us floor; the DMA stream runs
    ~98us busy, plus ~6us fixed runtime prologue + ~4us exit barrier.
  * Q and K are transposed on the HOST into the packed-pair layout
    [head, 64*(ball%2)+d, ball//2, seq] so the kernel needs NO PE
    transposes and no PSUM->SBUF transpose copies (v1's PE was 84% busy
    and HAM-throttled 75% of the time largely due to transpose-mode ops).
    Every DMA line is fully contiguous per partition.
  * V is repacked host-side to [head, seq, ball, 65] with a ones column
    baked in at d=64: softmax denominators fall out of the O matmul.
  * S^T per ball via one K=64 matmul; even/odd balls sit in PE row
    groups (0,0)/(64,0) (auto-derived from base_partition) and execute
    concurrently in the array.
  * exp(S/8) on ACT in one instruction per 8 balls (N=1024) to amortize
    the ~293ns fixed ACT instruction cost; S and O blocks are software-
    pipelined one set deep so the O matmuls never wait on ACT.
  * O = E^T @ [V|1] per ball; normalize on DVE via reciprocal+broadcast
    multiply; store fp16, host upcasts.
  * PSUM bank discipline: matmul outputs alternate banks; o_ps slot
    stride is padded to 128 floats so no output crosses a 2KB bank
    boundary (a 65-float stride silently corrupts slot 3).
  * Overlap: loads chunked (4x on head 0, 2x after) on the sync HWDGE
    ring, stores streamed per 4 sets on the gpsimd SWDGE ring (sync-ring
    stores measured far slower from FIFO coupling with loads); the last
    head tapers to 4-ball sets and finer stores to shorten the tail.
"""

import os
import sys

for _p in ("/opt/trn_rl_repo",):
    if _p not in sys.path and os.path.isdir(_p):
        sys.path.insert(0, _p)

from contextlib import ExitStack

import numpy as np

import concourse.bass as bass
import concourse.mybir as mybir
import concourse.tile as tile
from concourse import bacc
from concourse._compat import with_exitstack

B, H, N, DH = 4, 16, 8192, 64
BS = 128                 # ball size == SBUF partition count
NCORES = 8
HEADS = B * H // NCORES  # heads per core (8)
M = N // BS              # balls per head (64)
PAIRS = M // 2           # packed ball pairs (32)

FP32 = mybir.dt.float32
FP16 = mybir.dt.float16

SETB = 8                 # balls per pipeline set (one ACT exp instr each)
NSETS = M // SETB
IO_BUFS = int(os.environ.get("BALL_IO_BUFS", "4"))
NCHUNK = int(os.environ.get("BALL_NCHUNK", "4"))      # load chunks, head 0
NCHUNK_REST = int(os.environ.get("BALL_NCHUNK_REST", "2"))  # load chunks, heads 1+
STORE_SETS = int(os.environ.get("BALL_STORE_SETS", "8"))  # sets per store DMA
STORE_ENG = os.environ.get("BALL_STORE_ENG", "gpsimd")    # gpsimd (SWDGE) | sync (HWDGE)
TAIL_TAPER = os.environ.get("BALL_TAIL_TAPER", "1") == "1"
PAR_PROLOGUE = os.environ.get("BALL_PAR_PROLOGUE", "0") == "1"
GRPH = int(os.environ.get("BALL_GRPH", "1"))  # heads per load-group/DMA
V_ENG = os.environ.get("BALL_V_ENG", "sync")  # sync | gpsimd: V-load DGE queue


@with_exitstack
def ball_attention_kernel(
    ctx: ExitStack,
    tc: tile.TileContext,
    out_ap: bass.AP,
    q_ap: bass.AP,
    k_ap: bass.AP,
    v_ap: bass.AP,
    heads: int = HEADS,
    m: int = M,
):
    nc = tc.nc
    scale = 1.0 / float(np.sqrt(DH))

    # group tiles hold GRPH heads: bigger DMAs (2.1MB at GRPH=2) sit higher
    # on the DMA size-efficiency curve and halve trigger/completion count
    io_bufs = max(2, IO_BUFS // GRPH)
    io_pool = ctx.enter_context(tc.tile_pool(name="io", bufs=io_bufs))
    e_pool = ctx.enter_context(tc.tile_pool(name="e", bufs=2))
    r_pool = ctx.enter_context(tc.tile_pool(name="r", bufs=2))
    s_ps_pool = ctx.enter_context(tc.tile_pool(name="s_ps", bufs=2, space="PSUM"))
    o_ps_pool = ctx.enter_context(tc.tile_pool(name="o_ps", bufs=2, space="PSUM"))

    nsets = m // SETB

    for g in range(heads // GRPH):
        h0 = g * GRPH
        # ---- loads: per-partition lines are fully contiguous in HBM ----
        qt = io_pool.tile([BS, GRPH, PAIRS, BS], FP16, tag="qt")  # [64b+d, h, pair, seq]
        kt = io_pool.tile([BS, GRPH, PAIRS, BS], FP16, tag="kt")
        vt = io_pool.tile([BS, GRPH, m, DH + 1], FP16, tag="vt")  # [seq, h, ball, d|1]
        ob = io_pool.tile([BS, GRPH, m, DH], FP16, tag="ob")
        if g == 0:
            # pipeline-fill group: chunked per-head loads so compute on the
            # first pairs starts before the rest arrives (Tile tracks
            # subregion deps)
            for hh in range(GRPH):
                nch = NCHUNK if hh == 0 else NCHUNK_REST
                pc, mc = PAIRS // nch, m // nch
                for c in range(nch):
                    ps = slice(c * pc, (c + 1) * pc)
                    ms = slice(c * mc, (c + 1) * mc)
                    nc.sync.dma_start(qt[:, hh, ps, :], q_ap[h0 + hh][:, ps, :])
                    nc.sync.dma_start(kt[:, hh, ps, :], k_ap[h0 + hh][:, ps, :])
                    nc.sync.dma_start(vt[:, hh, ms, :], v_ap[h0 + hh][:, ms, :])
        elif GRPH > 1:
            # steady state: one multi-head transfer per tensor
            nc.sync.dma_start(qt, q_ap[h0 : h0 + GRPH].rearrange("g p a s -> p g a s"))
            nc.sync.dma_start(kt, k_ap[h0 : h0 + GRPH].rearrange("g p a s -> p g a s"))
            nc.sync.dma_start(vt, v_ap[h0 : h0 + GRPH].rearrange("g p a s -> p g a s"))
        else:
            # GRPH=1 steady state: per-head chunked loads (measured config)
            pc, mc = PAIRS // NCHUNK_REST, m // NCHUNK_REST
            for c in range(NCHUNK_REST):
                ps = slice(c * pc, (c + 1) * pc)
                ms = slice(c * mc, (c + 1) * mc)
                nc.sync.dma_start(qt[:, 0, ps, :], q_ap[h0][:, ps, :])
                nc.sync.dma_start(kt[:, 0, ps, :], k_ap[h0][:, ps, :])
                v_eng = nc.gpsimd if V_ENG == "gpsimd" else nc.sync
                v_eng.dma_start(vt[:, 0, ms, :], v_ap[h0][:, ms, :])

        for hh in range(GRPH):
            h = h0 + hh
            _per_head(
                tc, out_ap, qt, kt, vt, ob, hh, h, heads, m, nsets, scale,
                e_pool, r_pool, s_ps_pool, o_ps_pool,
            )


def _per_head(
    tc, out_ap, qt, kt, vt, ob, hh, h, heads, m, nsets, scale,
    e_pool, r_pool, s_ps_pool, o_ps_pool,
):
    nc = tc.nc
    if True:
        def do_o(s0, nb, e_sb, ob=ob, vt=vt):
            # O_unnorm = E^T @ [V|1]; ball j -> PSUM bank j%2 slot j//2.
            # Slot stride padded to 128 floats so every matmul output stays
            # inside one 2KB bank (65-float slots would cross at slot 3).
            o_ps = o_ps_pool.tile([BS, 2, SETB // 2, BS], FP32, tag="o")
            for j in range(nb):
                nc.tensor.matmul(
                    o_ps[:, j % 2, j // 2, 0 : DH + 1],
                    e_sb[:, j % 2, j // 2, :],
                    vt[:, hh, s0 + j, :],
                    start=True,
                    stop=True,
                )
            # normalize by the ones-column sums
            r_sb = r_pool.tile([BS, SETB], FP32, tag="r")
            half = nb // 2
            nc.vector.reciprocal(r_sb[:, 0:nb], o_ps[:, :, 0:half, DH])
            for b in range(2):
                nc.vector.tensor_mul(
                    ob[:, hh, s0 + b : s0 + nb : 2, :],
                    o_ps[:, b, 0:half, 0:DH],
                    r_sb[:, half * b : half * b + half]
                    .unsqueeze(2)
                    .broadcast_to([BS, half, DH]),
                )

        # Last head: no loads remain, so its stores can ride the sync HWDGE
        # ring (no FIFO coupling with loads, and ~1.5us faster completion
        # receipt than SWDGE — the exit barrier waits on the final store).
        # Finer stores there too, so the final transfer is small.
        if h < heads - 1:
            st_balls = SETB * STORE_SETS
            st_eng = {"gpsimd": nc.gpsimd, "sync": nc.sync,
                      "scalar": nc.scalar}[STORE_ENG]
        else:
            st_balls = SETB
            st_eng = nc.sync
        stored = [0]

        def store_upto(done):
            # stream out finished balls so the final store tail is short
            if done - stored[0] >= st_balls or (done == m and done > stored[0]):
                st_eng.dma_start(
                    out_ap[h][:, stored[0] : done, :], ob[:, hh, stored[0] : done, :]
                )
                stored[0] = done

        # set schedule: 8-ball sets; the final head tapers to 4-ball sets
        # to shorten the serial S->exp->O->norm->store chain at kernel end
        if h == heads - 1 and SETB == 8 and TAIL_TAPER:
            sched = [8] * (nsets - 1) + [4, 4]
        else:
            sched = [8] * nsets

        pend = None
        s0 = 0
        for nb in sched:
            # S^T matmuls: ball j contracts over its 64 d-partitions
            # (parity b -> partitions 64b, PE row group auto-derived).
            # Consecutive matmuls alternate PSUM banks (bank j%2).
            s_ps = s_ps_pool.tile([BS, 2, SETB // 2, BS], FP32, tag="s")
            for j in range(nb):
                ball = s0 + j
                pair, par = ball >> 1, ball & 1
                lo = 64 * par
                nc.tensor.matmul(
                    s_ps[:, j % 2, j // 2, :],
                    kt[lo : lo + 64, hh, pair, :],
                    qt[lo : lo + 64, hh, pair, :],
                    start=True,
                    stop=True,
                )
            # E = exp(S^T/8): one ACT op over both banks (N=128*nb)
            e_sb = e_pool.tile([BS, 2, SETB // 2, BS], FP16, tag="e")
            nc.scalar.activation(
                e_sb[:, :, 0 : nb // 2, :],
                s_ps[:, :, 0 : nb // 2, :],
                mybir.ActivationFunctionType.Exp,
                scale=scale,
            )
            # software pipeline: O for the previous set runs while this
            # set's exp is on ACT
            if pend is not None:
                do_o(*pend)
                store_upto(pend[0] + pend[1])
            pend = (s0, nb, e_sb)
            s0 += nb
        do_o(*pend)
        store_upto(m)


def build_nc(heads: int = HEADS, m: int = M):
    nc = bacc.Bacc("TRN2", target_bir_lowering=False, debug=False, num_devices=NCORES)
    q = nc.dram_tensor("q", [heads, BS, PAIRS, BS], FP16, kind="ExternalInput").ap()
    k = nc.dram_tensor("k", [heads, BS, PAIRS, BS], FP16, kind="ExternalInput").ap()
    v = nc.dram_tensor("v", [heads, BS, m, DH + 1], FP16, kind="ExternalInput").ap()
    o = nc.dram_tensor("out", [heads, BS, m, DH], FP16, kind="ExternalOutput").ap()
    with tile.TileContext(nc) as tc:
        ball_attention_kernel(tc, o, q, k, v, heads=heads, m=m)
    nc.compile()
    return nc


_NC_CACHE = {}


def _pack_qk(x: np.ndarray) -> np.ndarray:
    """[64, N, DH] fp32 -> [64, 128(=64*par+d), 32 pair, 128 seq] fp16."""
    xh = x.astype(np.float16)
    xh = xh.reshape(B * H, PAIRS, 2, BS, DH)          # h, pair, par, s, d
    xh = xh.transpose(0, 2, 4, 1, 3)                   # h, par, d, pair, s
    return np.ascontiguousarray(xh.reshape(B * H, BS, PAIRS, BS))


def _pack_v(x: np.ndarray) -> np.ndarray:
    """[64, N, DH] fp32 -> [64, 128 seq, 64 ball, 65] fp16 with ones col."""
    xh = x.astype(np.float16)
    xh = xh.reshape(B * H, M, BS, DH).transpose(0, 2, 1, 3)  # h, s, ball, d
    out = np.empty((B * H, BS, M, DH + 1), dtype=np.float16)
    out[..., :DH] = xh
    out[..., DH] = np.float16(1.0)
    return out


def kernel(q: np.ndarray, k: np.ndarray, v: np.ndarray) -> np.ndarray:
    from concourse.bass_utils import run_bass_kernel_spmd

    assert q.shape == (B, H, N, DH)
    if "nc" not in _NC_CACHE:
        _NC_CACHE["nc"] = build_nc()
    nc = _NC_CACHE["nc"]

    qt = _pack_qk(np.asarray(q, dtype=np.float32).reshape(B * H, N, DH))
    kt = _pack_qk(np.asarray(k, dtype=np.float32).reshape(B * H, N, DH))
    vt = _pack_v(np.asarray(v, dtype=np.float32).reshape(B * H, N, DH))
    hpc = HEADS
    in_maps = [
        {
            "q": np.ascontiguousarray(qt[c * hpc : (c + 1) * hpc]),
            "k": np.ascontiguousarray(kt[c * hpc : (c + 1) * hpc]),
            "v": np.ascontiguousarray(vt[c * hpc : (c + 1) * hpc]),
        }
        for c in range(NCORES)
    ]
    res = run_bass_kernel_spmd(nc, in_maps, core_ids=list(range(NCORES)))
    out = np.concatenate([res.results[c]["out"] for c in range(NCORES)], axis=0)
    # [64, seq, ball, d] fp16 -> [B, H, N, DH] fp32
    out = out.transpose(0, 2, 1, 3).reshape(B, H, N, DH)
    return out.astype(np.float32)


# revision 28
# speedup vs baseline: 1.0073x; 1.0073x over previous
"""Ball attention (block-local attention, ball size 128) on 8 Trainium2 cores.

Reference computation (per (b,h) head, per ball of 128 consecutive tokens):
    S = Q K^T / sqrt(64);  P = softmax(S, axis=-1);  O = P V

Sharding: the 64 (b,h) heads are split 8-per-core (pure data parallel).

Memory-roofline design (363us baseline -> ~114us measured):
  * All device I/O in fp16 (HBM traffic 33.7 MB/core vs 67 MB in fp32;
    keeps ~7e-4 output error, far under the 2e-2 gate). 33.7 MB at the
    358 GB/s per-core HBM limit is a ~94us floor; the DMA stream runs
    ~98us busy, plus ~6us fixed runtime prologue + ~4us exit barrier.
  * Q and K are transposed on the HOST into the packed-pair layout
    [head, 64*(ball%2)+d, ball//2, seq] so the kernel needs NO PE
    transposes and no PSUM->SBUF transpose copies (v1's PE was 84% busy
    and HAM-throttled 75% of the time largely due to transpose-mode ops).
    Every DMA line is fully contiguous per partition.
  * V is repacked host-side to [head, seq, ball, 65] with a ones column
    baked in at d=64: softmax denominators fall out of the O matmul.
  * S^T per ball via one K=64 matmul; even/odd balls sit in PE row
    groups (0,0)/(64,0) (auto-derived from base_partition) and execute
    concurrently in the array.
  * exp(S/8) on ACT in one instruction per 8 balls (N=1024) to amortize
    the ~293ns fixed ACT instruction cost; S and O blocks are software-
    pipelined one set deep so the O matmuls never wait on ACT.
  * O = E^T @ [V|1] per ball; normalize on DVE via reciprocal+broadcast
    multiply; store fp16, host upcasts.
  * PSUM bank discipline: matmul outputs alternate banks; o_ps slot
    stride is padded to 128 floats so no output crosses a 2KB bank
    boundary (a 65-float stride silently corrupts slot 3).
  * Overlap: loads chunked (4x on head 0, 2x after) on the sync HWDGE
    ring, stores streamed per 4 sets on the gpsimd SWDGE ring (sync-ring
    stores measured far slower from FIFO coupling with loads); the last
    head tapers to 4-ball sets and finer stores to shorten the tail.
"""

import os
import sys

for _p in ("/opt/trn_rl_repo",):
    if _p not in sys.path and os.path.isdir(_p):
        sys.path.insert(0, _p)

from contextlib import ExitStack

import numpy as np

import concourse.bass as bass
import concourse.mybir as mybir
import concourse.tile as tile
from concourse import bacc
from concourse._compat import with_exitstack

B, H, N, DH = 4, 16, 8192, 64
BS = 128                 # ball size == SBUF partition count
NCORES = 8
HEADS = B * H // NCORES  # heads per core (8)
M = N // BS              # balls per head (64)
PAIRS = M // 2           # packed ball pairs (32)

FP32 = mybir.dt.float32
FP16 = mybir.dt.float16

SETB = 8                 # balls per pipeline set (one ACT exp instr each)
NSETS = M // SETB
IO_BUFS = int(os.environ.get("BALL_IO_BUFS", "4"))
NCHUNK = int(os.environ.get("BALL_NCHUNK", "4"))      # load chunks, head 0
NCHUNK_REST = int(os.environ.get("BALL_NCHUNK_REST", "2"))  # load chunks, heads 1+
STORE_SETS = int(os.environ.get("BALL_STORE_SETS", "8"))  # sets per store DMA
STORE_ENG = os.environ.get("BALL_STORE_ENG", "gpsimd")    # gpsimd (SWDGE) | sync (HWDGE)
TAIL_TAPER = os.environ.get("BALL_TAIL_TAPER", "1") == "1"
PAR_PROLOGUE = os.environ.get("BALL_PAR_PROLOGUE", "0") == "1"
GRPH = int(os.environ.get("BALL_GRPH", "1"))  # heads per load-group/DMA
V_ENG = os.environ.get("BALL_V_ENG", "sync")  # sync | gpsimd: V-load DGE queue
TAIL_TAPER2 = os.environ.get("BALL_TAIL_TAPER2", "0") == "1"


@with_exitstack
def ball_attention_kernel(
    ctx: ExitStack,
    tc: tile.TileContext,
    out_ap: bass.AP,
    q_ap: bass.AP,
    k_ap: bass.AP,
    v_ap: bass.AP,
    heads: int = HEADS,
    m: int = M,
):
    nc = tc.nc
    scale = 1.0 / float(np.sqrt(DH))

    # group tiles hold GRPH heads: bigger DMAs (2.1MB at GRPH=2) sit higher
    # on the DMA size-efficiency curve and halve trigger/completion count
    io_bufs = max(2, IO_BUFS // GRPH)
    io_pool = ctx.enter_context(tc.tile_pool(name="io", bufs=io_bufs))
    e_pool = ctx.enter_context(tc.tile_pool(name="e", bufs=2))
    r_pool = ctx.enter_context(tc.tile_pool(name="r", bufs=2))
    s_ps_pool = ctx.enter_context(tc.tile_pool(name="s_ps", bufs=2, space="PSUM"))
    o_ps_pool = ctx.enter_context(tc.tile_pool(name="o_ps", bufs=2, space="PSUM"))

    nsets = m // SETB

    for g in range(heads // GRPH):
        h0 = g * GRPH
        # ---- loads: per-partition lines are fully contiguous in HBM ----
        qt = io_pool.tile([BS, GRPH, PAIRS, BS], FP16, tag="qt")  # [64b+d, h, pair, seq]
        kt = io_pool.tile([BS, GRPH, PAIRS, BS], FP16, tag="kt")
        vt = io_pool.tile([BS, GRPH, m, DH + 1], FP16, tag="vt")  # [seq, h, ball, d|1]
        ob = io_pool.tile([BS, GRPH, m, DH], FP16, tag="ob")
        if g == 0:
            # pipeline-fill group: chunked per-head loads so compute on the
            # first pairs starts before the rest arrives (Tile tracks
            # subregion deps)
            for hh in range(GRPH):
                nch = NCHUNK if hh == 0 else NCHUNK_REST
                pc, mc = PAIRS // nch, m // nch
                for c in range(nch):
                    ps = slice(c * pc, (c + 1) * pc)
                    ms = slice(c * mc, (c + 1) * mc)
                    nc.sync.dma_start(qt[:, hh, ps, :], q_ap[h0 + hh][:, ps, :])
                    nc.sync.dma_start(kt[:, hh, ps, :], k_ap[h0 + hh][:, ps, :])
                    nc.sync.dma_start(vt[:, hh, ms, :], v_ap[h0 + hh][:, ms, :])
        elif GRPH > 1:
            # steady state: one multi-head transfer per tensor
            nc.sync.dma_start(qt, q_ap[h0 : h0 + GRPH].rearrange("g p a s -> p g a s"))
            nc.sync.dma_start(kt, k_ap[h0 : h0 + GRPH].rearrange("g p a s -> p g a s"))
            nc.sync.dma_start(vt, v_ap[h0 : h0 + GRPH].rearrange("g p a s -> p g a s"))
        else:
            # GRPH=1 steady state: per-head chunked loads (measured config)
            pc, mc = PAIRS // NCHUNK_REST, m // NCHUNK_REST
            for c in range(NCHUNK_REST):
                ps = slice(c * pc, (c + 1) * pc)
                ms = slice(c * mc, (c + 1) * mc)
                nc.sync.dma_start(qt[:, 0, ps, :], q_ap[h0][:, ps, :])
                nc.sync.dma_start(kt[:, 0, ps, :], k_ap[h0][:, ps, :])
                v_eng = nc.gpsimd if V_ENG == "gpsimd" else nc.sync
                v_eng.dma_start(vt[:, 0, ms, :], v_ap[h0][:, ms, :])

        for hh in range(GRPH):
            h = h0 + hh
            _per_head(
                tc, out_ap, qt, kt, vt, ob, hh, h, heads, m, nsets, scale,
                e_pool, r_pool, s_ps_pool, o_ps_pool,
            )


def _per_head(
    tc, out_ap, qt, kt, vt, ob, hh, h, heads, m, nsets, scale,
    e_pool, r_pool, s_ps_pool, o_ps_pool,
):
    nc = tc.nc
    if True:
        def do_o(s0, nb, e_sb, ob=ob, vt=vt):
            # O_unnorm = E^T @ [V|1]; ball j -> PSUM bank j%2 slot j//2.
            # Slot stride padded to 128 floats so every matmul output stays
            # inside one 2KB bank (65-float slots would cross at slot 3).
            o_ps = o_ps_pool.tile([BS, 2, SETB // 2, BS], FP32, tag="o")
            for j in range(nb):
                nc.tensor.matmul(
                    o_ps[:, j % 2, j // 2, 0 : DH + 1],
                    e_sb[:, j % 2, j // 2, :],
                    vt[:, hh, s0 + j, :],
                    start=True,
                    stop=True,
                )
            # normalize by the ones-column sums
            r_sb = r_pool.tile([BS, SETB], FP32, tag="r")
            half = nb // 2
            nc.vector.reciprocal(r_sb[:, 0:nb], o_ps[:, :, 0:half, DH])
            for b in range(2):
                nc.vector.tensor_mul(
                    ob[:, hh, s0 + b : s0 + nb : 2, :],
                    o_ps[:, b, 0:half, 0:DH],
                    r_sb[:, half * b : half * b + half]
                    .unsqueeze(2)
                    .broadcast_to([BS, half, DH]),
                )

        # Last head: no loads remain, so its stores can ride the sync HWDGE
        # ring (no FIFO coupling with loads, and ~1.5us faster completion
        # receipt than SWDGE — the exit barrier waits on the final store).
        # Finer stores there too, so the final transfer is small.
        if h < heads - 1:
            st_balls = SETB * STORE_SETS
            st_eng = {"gpsimd": nc.gpsimd, "sync": nc.sync,
                      "scalar": nc.scalar}[STORE_ENG]
        else:
            st_balls = SETB
            st_eng = nc.sync
        stored = [0]

        def store_upto(done):
            # stream out finished balls so the final store tail is short
            if done - stored[0] >= st_balls or (done == m and done > stored[0]):
                st_eng.dma_start(
                    out_ap[h][:, stored[0] : done, :], ob[:, hh, stored[0] : done, :]
                )
                stored[0] = done

        # set schedule: 8-ball sets; the final head tapers to 4-ball sets
        # to shorten the serial S->exp->O->norm->store chain at kernel end
        if h == heads - 1 and SETB == 8 and TAIL_TAPER:
            # TAPER2: final sets 4,2,2 — even shorter end-of-kernel chain
            sched = (
                [8] * (nsets - 1) + [4, 2, 2]
                if TAIL_TAPER2
                else [8] * (nsets - 1) + [4, 4]
            )
        else:
            sched = [8] * nsets

        pend = None
        s0 = 0
        for nb in sched:
            # S^T matmuls: ball j contracts over its 64 d-partitions
            # (parity b -> partitions 64b, PE row group auto-derived).
            # Consecutive matmuls alternate PSUM banks (bank j%2).
            s_ps = s_ps_pool.tile([BS, 2, SETB // 2, BS], FP32, tag="s")
            for j in range(nb):
                ball = s0 + j
                pair, par = ball >> 1, ball & 1
                lo = 64 * par
                nc.tensor.matmul(
                    s_ps[:, j % 2, j // 2, :],
                    kt[lo : lo + 64, hh, pair, :],
                    qt[lo : lo + 64, hh, pair, :],
                    start=True,
                    stop=True,
                )
            # E = exp(S^T/8): one ACT op over both banks (N=128*nb)
            e_sb = e_pool.tile([BS, 2, SETB // 2, BS], FP16, tag="e")
            nc.scalar.activation(
                e_sb[:, :, 0 : nb // 2, :],
                s_ps[:, :, 0 : nb // 2, :],
                mybir.ActivationFunctionType.Exp,
                scale=scale,
            )
            # software pipeline: O for the previous set runs while this
            # set's exp is on ACT
            if pend is not None:
                do_o(*pend)
                store_upto(pend[0] + pend[1])
            pend = (s0, nb, e_sb)
            s0 += nb
        do_o(*pend)
        store_upto(m)


def build_nc(heads: int = HEADS, m: int = M):
    nc = bacc.Bacc("TRN2", target_bir_lowering=False, debug=False, num_devices=NCORES)
    q = nc.dram_tensor("q", [heads, BS, PAIRS, BS], FP16, kind="ExternalInput").ap()
    k = nc.dram_tensor("k", [heads, BS, PAIRS, BS], FP16, kind="ExternalInput").ap()
    v = nc.dram_tensor("v", [heads, BS, m, DH + 1], FP16, kind="ExternalInput").ap()
    o = nc.dram_tensor("out", [heads, BS, m, DH], FP16, kind="ExternalOutput").ap()
    with tile.TileContext(nc) as tc:
        ball_attention_kernel(tc, o, q, k, v, heads=heads, m=m)
    nc.compile()
    return nc


_NC_CACHE = {}


def _pack_qk(x: np.ndarray) -> np.ndarray:
    """[64, N, DH] fp32 -> [64, 128(=64*par+d), 32 pair, 128 seq] fp16."""
    xh = x.astype(np.float16)
    xh = xh.reshape(B * H, PAIRS, 2, BS, DH)          # h, pair, par, s, d
    xh = xh.transpose(0, 2, 4, 1, 3)                   # h, par, d, pair, s
    return np.ascontiguousarray(xh.reshape(B * H, BS, PAIRS, BS))


def _pack_v(x: np.ndarray) -> np.ndarray:
    """[64, N, DH] fp32 -> [64, 128 seq, 64 ball, 65] fp16 with ones col."""
    xh = x.astype(np.float16)
    xh = xh.reshape(B * H, M, BS, DH).transpose(0, 2, 1, 3)  # h, s, ball, d
    out = np.empty((B * H, BS, M, DH + 1), dtype=np.float16)
    out[..., :DH] = xh
    out[..., DH] = np.float16(1.0)
    return out


def kernel(q: np.ndarray, k: np.ndarray, v: np.ndarray) -> np.ndarray:
    from concourse.bass_utils import run_bass_kernel_spmd

    assert q.shape == (B, H, N, DH)
    if "nc" not in _NC_CACHE:
        _NC_CACHE["nc"] = build_nc()
    nc = _NC_CACHE["nc"]

    qt = _pack_qk(np.asarray(q, dtype=np.float32).reshape(B * H, N, DH))
    kt = _pack_qk(np.asarray(k, dtype=np.float32).reshape(B * H, N, DH))
    vt = _pack_v(np.asarray(v, dtype=np.float32).reshape(B * H, N, DH))
    hpc = HEADS
    in_maps = [
        {
            "q": np.ascontiguousarray(qt[c * hpc : (c + 1) * hpc]),
            "k": np.ascontiguousarray(kt[c * hpc : (c + 1) * hpc]),
            "v": np.ascontiguousarray(vt[c * hpc : (c + 1) * hpc]),
        }
        for c in range(NCORES)
    ]
    res = run_bass_kernel_spmd(nc, in_maps, core_ids=list(range(NCORES)))
    out = np.concatenate([res.results[c]["out"] for c in range(NCORES)], axis=0)
    # [64, seq, ball, d] fp16 -> [B, H, N, DH] fp32
    out = out.transpose(0, 2, 1, 3).reshape(B, H, N, DH)
    return out.astype(np.float32)
